# revision 1
# baseline (speedup 1.0000x reference)
import os
os.environ.setdefault("JAX_PLATFORMS", "cpu")

import numpy as np
import jax
import jax.numpy as jnp
from functools import partial

N, T, E, H, C, OWN, TAPS, L, OUT = 50000, 50000, 500000, 256, 32, 4, 4, 2, 2


def _seg(x, idx, n):
    return jax.ops.segment_sum(x, idx, num_segments=n)


@partial(jax.jit, backend="cpu")
def _compute(own_obs, agent_pos, target_pos, agent_edges, t_src, t_dst,
             gnn_edges, edge_attr, phiA_W, phiA_b, rhoA_W, rhoA_b,
             phiT_W, phiT_b, rhoT_W, rhoT_b, rin_W, rin_b, gnn_W, gnn_b,
             rout_W, rout_b):
    n = agent_pos.shape[0]
    lrelu = lambda v: jax.nn.leaky_relu(v, 0.01)

    e0, e1 = agent_edges[0], agent_edges[1]
    h = (agent_pos[e0] - agent_pos[e1]) @ phiA_W + phiA_b
    a = _seg(h, e1, n) @ rhoA_W + rhoA_b

    ht = (target_pos[t_src] - agent_pos[t_dst]) @ phiT_W + phiT_b
    t = _seg(ht, t_dst, n) @ rhoT_W + rhoT_b

    x = jnp.concatenate([own_obs, a, t], axis=1) @ rin_W + rin_b
    g0, g1 = gnn_edges[0], gnn_edges[1]
    w = edge_attr[:, None]
    for l in range(gnn_W.shape[0]):
        z = x @ gnn_W[l, 0]
        s = x
        for k in range(1, gnn_W.shape[1]):
            s = _seg(w * s[g0], g1, n)
            z = z + s @ gnn_W[l, k]
        x = lrelu(z + gnn_b[l])
    return x @ rout_W + rout_b


def kernel(**inputs):
    args = {}
    for k, v in inputs.items():
        v = np.asarray(v)
        if v.dtype == np.int64:
            v = v.astype(np.int32)
        args[k] = v
    out = _compute(
        args["own_obs"], args["agent_pos"], args["target_pos"],
        args["agent_edges"], args["t_src"], args["t_dst"],
        args["gnn_edges"], args["edge_attr"],
        args["phiA_W"], args["phiA_b"], args["rhoA_W"], args["rhoA_b"],
        args["phiT_W"], args["phiT_b"], args["rhoT_W"], args["rhoT_b"],
        args["rin_W"], args["rin_b"], args["gnn_W"], args["gnn_b"],
        args["rout_W"], args["rout_b"],
    )
    return np.asarray(out, dtype=np.float32)



# revision 22
# speedup vs baseline: 101.8448x; 101.8448x over previous
"""GCN+DeepSet Trainium2 kernel (8 NeuronCores, SPMD).

The reference DeepSets have no nonlinearity, so they collapse to linear maps
of per-node segment sums:
  a_i = (sumA_i - degA_i*pos_i) @ MA + degA_i*cA + rhoA_b,   MA = phiA_W@rhoA_W
  t_i analogous with target edges.
  x0  = [own|a|t] @ rin_W + rin_b = f @ G + g0 with
  f_i = [own(4), uA(2), degA(1), uT(2), degT(1), 0...],  uA = sumA - degA*pos.

GCN layer (Horner):  u_T = x@W_T;  u_k = S u_{k+1} + x@W_k;  z = u_0.
S is applied as: DMA indirect gather of node rows from a replicated DRAM
table, then one-hot scatter matmuls on the PE accumulating into PSUM.
One-hots (dst-window x edge-tile, edge weight folded in) are built on the
host; tile->window maps are shared across cores (per-window tile budget =
max over cores), so a single SPMD program serves all 8 cores.

Sharding: core c owns nodes [c*NS, (c+1)*NS) (padded to NSP = 128*nslots).
Per tap the updated shard is AllGather'd into the full bf16 node table.
"""

import os
os.environ.setdefault("JAX_PLATFORMS", "axon,cpu")

import sys
if "/opt/trn_rl_repo" not in sys.path:
    sys.path.insert(0, "/opt/trn_rl_repo")

import numpy as np
import ml_dtypes

BF16 = ml_dtypes.bfloat16


class Cfg:
    def __init__(self, N=50000, T=50000, E=500000, NCORES=8, TAPS=4, L=2,
                 C=32, OWN=4, OUT=2, WIN=32, KTILE=128, GCHUNK=32, DSCHUNK=64):
        self.N, self.T, self.E, self.NCORES = N, T, E, NCORES
        self.TAPS, self.L, self.C, self.OWN, self.OUT = TAPS, L, C, OWN, OUT
        self.WIN, self.KTILE, self.GCHUNK, self.DSCHUNK = WIN, KTILE, GCHUNK, DSCHUNK
        self.NS = N // NCORES                       # real nodes per core
        self.nslots = (self.NS + 127) // 128
        self.NSP = self.nslots * 128                # padded nodes per core
        self.nwin = self.NSP // WIN
        assert self.nwin % 4 == 0


CFG = Cfg()


# ---------------------------------------------------------------------------
# Host-side preprocessing
# ---------------------------------------------------------------------------

IDX_SPLIT = 32768


def _shared_budget_pack(cfg, per_core_edges, split_src=False):
    """per_core_edges: list of (dst_local, src_padded_global, w) per core.
    Tiles are grouped by (src_half if split_src, dst window); per-group tile
    budget = max over cores.  Returns (winid [nT], nT, nT_low, packed) with
    per-core packed = (oh [128,nT,WIN], gidx16 [nT*KT] int16 local-to-half,
    slot, src_sorted)."""
    KT, WIN, nwin = cfg.KTILE, cfg.WIN, cfg.nwin
    nhalf = 2 if split_src else 1
    ngrp = nhalf * nwin

    def grp(dl, sg):
        g = dl // WIN
        if split_src:
            g = (sg >= IDX_SPLIT) * nwin + g
        return g

    counts = np.zeros((cfg.NCORES, ngrp), np.int64)
    for c, (dl, sg, _) in enumerate(per_core_edges):
        counts[c] = np.bincount(grp(dl, sg), minlength=ngrp)
    budget = -(-counts.max(0) // KT)
    grpid = np.repeat(np.arange(ngrp), budget)
    winid = (grpid % nwin).astype(np.int32)
    nT = int(budget.sum())
    nT_low = int(budget[:nwin].sum()) if split_src else nT
    tstart = np.concatenate([[0], np.cumsum(budget)])

    packed = []
    for c, (dl, sg, w) in enumerate(per_core_edges):
        g = grp(dl, sg)
        order = np.argsort(g, kind="stable")
        dl, sg, w, g = dl[order], sg[order], w[order], g[order]
        estart = np.concatenate([[0], np.cumsum(counts[c])])
        slot = np.zeros(dl.shape[0], np.int64)
        for gi in range(ngrp):
            cnt = counts[c, gi]
            if cnt:
                slot[estart[gi]:estart[gi] + cnt] = \
                    tstart[gi] * KT + np.arange(cnt)
        oh = np.zeros((nT * KT, WIN), np.float32)
        oh[slot, dl - winid[slot // KT] * WIN] = w
        gidx = np.zeros(nT * KT, np.int64)
        gidx[slot] = sg - (sg >= IDX_SPLIT) * IDX_SPLIT if split_src else sg
        packed.append((oh.reshape(nT, KT, WIN).transpose(1, 0, 2).copy(),
                       gidx.astype(np.int16), slot, sg))
    return winid, nT, nT_low, packed


def gather_chunks(cfg, nT_low, nT):
    """(t0, tn, half) chunk list; chunks never straddle the half boundary."""
    ch = []
    for t0 in range(0, nT_low, cfg.GCHUNK):
        ch.append((t0, min(cfg.GCHUNK, nT_low - t0), 0))
    for t0 in range(nT_low, nT, cfg.GCHUNK):
        ch.append((t0, min(cfg.GCHUNK, nT - t0), 1))
    return ch


def _wrap16(idx_flat, cfg, chunks):
    """Wrapped idx buffer [128, nT*8] int16: chunk (t0, tn) occupies columns
    [t0*8, (t0+tn)*8); within a chunk slot i sits at [i % 16, i // 16],
    replicated over the 8 groups of 16 partitions."""
    KT = cfg.KTILE
    nT = idx_flat.shape[0] // KT
    out = np.zeros((128, nT * 8), np.int16)
    for (t0, tn, _) in chunks:
        blk = idx_flat[t0 * KT:(t0 + tn) * KT]          # [tn*128]
        wrapped = blk.reshape(-1, 16).T                  # [16, tn*8]
        out[:, t0 * 8:(t0 + tn) * 8] = np.tile(wrapped, (8, 1))
    return out


def _prep(cfg, inputs):
    f32 = np.float32
    g = lambda k: np.asarray(inputs[k], f32)
    pA_W, pA_b, rA_W, rA_b = g("phiA_W"), g("phiA_b"), g("rhoA_W"), g("rhoA_b")
    pT_W, pT_b, rT_W, rT_b = g("phiT_W"), g("phiT_b"), g("rhoT_W"), g("rhoT_b")
    rin_W, rin_b = g("rin_W"), g("rin_b")
    gnn_W, gnn_b = g("gnn_W"), g("gnn_b")
    rout_W, rout_b = g("rout_W"), g("rout_b")
    C, OWN = cfg.C, cfg.OWN

    MA, cA = pA_W @ rA_W, pA_b @ rA_W
    MT, cT = pT_W @ rT_W, pT_b @ rT_W
    na = MA.shape[1]                                  # 6
    Gm = np.zeros((C, C), f32)
    Gm[0:OWN] = rin_W[0:OWN]
    Gm[OWN:OWN + 2] = MA @ rin_W[OWN:OWN + na]
    Gm[OWN + 2] = cA @ rin_W[OWN:OWN + na]
    Gm[OWN + 3:OWN + 5] = MT @ rin_W[OWN + na:OWN + 2 * na]
    Gm[OWN + 5] = cT @ rin_W[OWN + na:OWN + 2 * na]
    g0 = rin_b + rA_b @ rin_W[OWN:OWN + na] + rT_b @ rin_W[OWN + na:OWN + 2 * na]

    ae = np.asarray(inputs["agent_edges"]).astype(np.int64)
    ts = np.asarray(inputs["t_src"]).astype(np.int64)
    td = np.asarray(inputs["t_dst"]).astype(np.int64)
    ge = np.asarray(inputs["gnn_edges"]).astype(np.int64)
    ea = g("edge_attr")
    agent_pos, target_pos, own_obs = g("agent_pos"), g("target_pos"), g("own_obs")

    NS, NSP = cfg.NS, cfg.NSP

    def split(dst, src, w):
        per = []
        for c in range(cfg.NCORES):
            m = (dst >= c * NS) & (dst < (c + 1) * NS)
            per.append((dst[m] - c * NS, src[m], w[m]))
        return per

    # gnn: src mapped to padded-global id
    gsrc_pad = (ge[0] // NS) * NSP + (ge[0] % NS)
    widG, nTG, nTG_low, packG = _shared_budget_pack(
        cfg, split(ge[1], gsrc_pad, ea), split_src=True)
    widA, nTA, _, packA = _shared_budget_pack(
        cfg, split(ae[1], ae[0], np.ones(ae.shape[1], f32)))
    widT, nTT, _, packT = _shared_budget_pack(
        cfg, split(td, ts, np.ones(td.shape[0], f32)))
    chunks = gather_chunks(cfg, nTG_low, nTG)

    cores = []
    for c in range(cfg.NCORES):
        lo = c * NS
        core = {}
        core["A_oh"] = packA[c][0].astype(BF16)
        core["T_oh"] = packT[c][0].astype(BF16)
        core["G_oh"] = packG[c][0].astype(BF16)
        core["G_idx"] = _wrap16(packG[c][1], cfg, chunks)

        def payload(pack, pos_tab, nT):
            _, _, slot, src_sorted = pack
            pay = np.zeros((nT * cfg.KTILE, 4), f32)
            pay[slot, 0:2] = pos_tab[src_sorted]
            pay[slot, 2] = 1.0
            return pay.reshape(nT, cfg.KTILE, 4).transpose(1, 0, 2).astype(BF16)

        core["A_pay"] = payload(packA[c], agent_pos, nTA)
        core["T_pay"] = payload(packT[c], target_pos, nTT)

        def shard(tab):
            ch = tab.shape[1]
            out = np.zeros((NSP, ch), f32)
            out[:NS] = tab[lo:lo + NS]
            return out.reshape(cfg.nslots, 128, ch).transpose(1, 0, 2).copy()

        core["own_obs"] = shard(own_obs)
        core["pos"] = shard(agent_pos)
        cores.append(core)

    Wst = gnn_W.transpose(2, 0, 1, 3).reshape(C, cfg.L * (cfg.TAPS + 1) * C)
    consts = dict(
        Gmat=np.tile(Gm, (4, 1)).astype(BF16),
        Wstack=np.tile(Wst, (4, 1)).astype(BF16),
        Wout=np.tile(rout_W, (4, 1)).astype(BF16),
        g0_bc=np.broadcast_to(g0, (128, cfg.nslots, C)).astype(BF16).copy(),
        b_bc=np.concatenate(
            [np.broadcast_to(gnn_b[l], (128, cfg.nslots, C))
             for l in range(cfg.L)], axis=1).astype(f32).copy(),
        rout_bc=np.broadcast_to(rout_b, (128, cfg.nslots, cfg.OUT))
        .astype(f32).copy(),
    )
    return dict(cores=cores, consts=consts, widA=widA, widT=widT, widG=widG,
                nTA=nTA, nTT=nTT, nTG=nTG, nTG_low=nTG_low, chunks=chunks)


# ---------------------------------------------------------------------------
# Device program
# ---------------------------------------------------------------------------

DEBUG = False
NO_COLLECTIVE = False


def build_program(cfg, prep):
    from concourse import bacc, tile
    import concourse.mybir as mybir
    from concourse.bass import IndirectOffsetOnAxis

    dt = mybir.dt
    C, OUT, WIN, KT = cfg.C, cfg.OUT, cfg.WIN, cfg.KTILE
    nslots, nwin = cfg.nslots, cfg.nwin
    nTA, nTT, nTG = prep["nTA"], prep["nTT"], prep["nTG"]
    widA, widT, widG = prep["widA"], prep["widT"], prep["widG"]
    chunks = prep["chunks"]
    NTAB = cfg.NCORES * cfg.NSP
    ROWP = 128  # table rows padded to 128 bf16 = 256B for dma_gather

    nc = bacc.Bacc("TRN2", target_bir_lowering=False, debug=False,
                   num_devices=cfg.NCORES)

    def din(name, shape, d):
        return nc.dram_tensor(name, list(shape), d, kind="ExternalInput").ap()

    A_oh = din("A_oh", (128, nTA, WIN), dt.bfloat16)
    A_pay = din("A_pay", (128, nTA, 4), dt.bfloat16)
    T_oh = din("T_oh", (128, nTT, WIN), dt.bfloat16)
    T_pay = din("T_pay", (128, nTT, 4), dt.bfloat16)
    G_ohd = din("G_oh", (128, nTG, WIN), dt.bfloat16)
    G_idx = din("G_idx", (128, nTG * 8), dt.int16)
    own_d = din("own_obs", (128, nslots, cfg.OWN), dt.float32)
    pos_d = din("pos", (128, nslots, 2), dt.float32)
    Gmat_d = din("Gmat", (4 * C, C), dt.bfloat16)
    Wst_d = din("Wstack", (4 * C, cfg.L * (cfg.TAPS + 1) * C), dt.bfloat16)
    Wout_d = din("Wout", (4 * C, OUT), dt.bfloat16)
    g0_d = din("g0_bc", (128, nslots, C), dt.bfloat16)
    b_d = din("b_bc", (128, cfg.L * nslots, C), dt.float32)
    rb_d = din("rout_bc", (128, nslots, OUT), dt.float32)
    out_sh = nc.dram_tensor("out_sh", [128, nslots, OUT], dt.float32,
                            kind="ExternalOutput").ap()
    dbg = {}
    if DEBUG:
        for nm in ("dbg_f", "dbg_x0", "dbg_u4", "dbg_u3"):
            dbg[nm] = nc.dram_tensor(nm, [128, nslots, C], dt.float32,
                                     kind="ExternalOutput").ap()
        dbg["dbg_xt"] = nc.dram_tensor("dbg_xt", [NTAB, ROWP], dt.bfloat16,
                                       kind="ExternalOutput").ap()
        dbg["dbg_y"] = nc.dram_tensor("dbg_y", [128, cfg.GCHUNK, ROWP],
                                      dt.bfloat16, kind="ExternalOutput").ap()

    xtab = nc.dram_tensor("xtab", [NTAB, ROWP], dt.bfloat16).ap()
    ush = nc.dram_tensor("ush", [cfg.NSP, ROWP], dt.bfloat16).ap()

    def win_ap(ps, widx, ncols, col0=0):
        return ps[32 * (widx % 4):32 * (widx % 4) + 32, widx // 4,
                  col0:col0 + ncols]

    AG = mybir.AluOpType

    with tile.TileContext(nc) as tc:
        with (
            tc.tile_pool(name="psum", bufs=2, space="PSUM") as pp,
            tc.tile_pool(name="sb", bufs=2) as sb,
            tc.tile_pool(name="cst", bufs=1) as cst,
        ):
            g_oh = cst.tile([128, nTG, WIN], dt.bfloat16, tag="goh")
            nc.sync.dma_start(out=g_oh[:], in_=G_ohd[:])
            gidx = cst.tile([128, nTG * 8], dt.int16, tag="gidx")
            nc.sync.dma_start(out=gidx[:], in_=G_idx[:])
            gmat = cst.tile([4 * C, C], dt.bfloat16, tag="gmat")
            nc.sync.dma_start(out=gmat[:], in_=Gmat_d[:])
            wst = cst.tile([4 * C, cfg.L * (cfg.TAPS + 1) * C], dt.bfloat16, tag="wst")
            nc.sync.dma_start(out=wst[:], in_=Wst_d[:])
            wout = cst.tile([4 * C, OUT], dt.bfloat16, tag="wout")
            nc.sync.dma_start(out=wout[:], in_=Wout_d[:])
            g0b = cst.tile([128, nslots, C], dt.bfloat16, tag="g0b")
            nc.sync.dma_start(out=g0b[:], in_=g0_d[:])
            bb = cst.tile([128, cfg.L * nslots, C], dt.float32, tag="bb")
            nc.sync.dma_start(out=bb[:], in_=b_d[:])
            rbb = cst.tile([128, nslots, OUT], dt.float32, tag="rbb")
            nc.sync.dma_start(out=rbb[:], in_=rb_d[:])
            ubf = cst.tile([128, nslots, ROWP], dt.bfloat16, tag="ubf")
            nc.vector.memset(ubf[:, :, C:ROWP], 0.0)

            def wk(l, k, q):
                o = (l * (cfg.TAPS + 1) + k) * C
                return wst[32 * q:32 * q + 32, o:o + C]

            # ---- Phase 1: deepset one-hot scatters -> ds psum table
            ds = pp.tile([128, 64, C], dt.float32, tag="u")
            q_marked = set()
            seen = set()
            for (nT, oh_d, pay_d, wid, col0) in (
                (nTA, A_oh, A_pay, widA, 0),
                (nTT, T_oh, T_pay, widT, 4),
            ):
                for t0 in range(0, nT, cfg.DSCHUNK):
                    tn = min(cfg.DSCHUNK, nT - t0)
                    ohb = sb.tile([128, cfg.DSCHUNK, WIN], dt.bfloat16, tag="dsoh")
                    nc.sync.dma_start(out=ohb[:, 0:tn], in_=oh_d[:, t0:t0 + tn])
                    pyb = sb.tile([128, cfg.DSCHUNK, 4], dt.bfloat16, tag="dspay")
                    nc.sync.dma_start(out=pyb[:, 0:tn], in_=pay_d[:, t0:t0 + tn])
                    for ti in range(tn):
                        w = int(wid[t0 + ti])
                        bank = (w // 4) // 16
                        st = (w % 4, bank) not in q_marked
                        q_marked.add((w % 4, bank))
                        seen.add((col0, w))
                        nc.tensor.matmul(win_ap(ds, w, 4, col0),
                                         ohb[:, ti], pyb[:, ti],
                                         start=st, stop=True,
                                         skip_group_check=True,
                                         tile_position=(0, 32 * (w % 4)))
            for col0 in (0, 4):
                for w in range(nwin):
                    if (col0, w) not in seen:
                        nc.vector.memset(win_ap(ds, w, 4, col0), 0.0)

            # ---- Phase 2: f = [own, uA, degA, uT, degT, 0...], bf16
            f_sb = sb.tile([128, nslots, C], dt.bfloat16, tag="fsb")
            own_sb = sb.tile([128, nslots, cfg.OWN], dt.float32, tag="own")
            nc.sync.dma_start(out=own_sb[:], in_=own_d[:])
            pos_sb = sb.tile([128, nslots, 2], dt.float32, tag="pos")
            nc.sync.dma_start(out=pos_sb[:], in_=pos_d[:])
            tmp = sb.tile([128, nslots, 2], dt.float32, tag="tmp")
            o = cfg.OWN
            nc.vector.tensor_copy(out=f_sb[:, :, 0:o], in_=own_sb[:])
            nc.vector.tensor_tensor(
                out=tmp[:], in0=ds[:, 0:nslots, 2:3].broadcast_to([128, nslots, 2]),
                in1=pos_sb[:], op=AG.mult)
            nc.vector.tensor_tensor(out=f_sb[:, :, o:o + 2], in0=ds[:, 0:nslots, 0:2],
                                    in1=tmp[:], op=AG.subtract)
            nc.vector.tensor_copy(out=f_sb[:, :, o + 2:o + 3], in_=ds[:, 0:nslots, 2:3])
            nc.vector.tensor_tensor(
                out=tmp[:], in0=ds[:, 0:nslots, 6:7].broadcast_to([128, nslots, 2]),
                in1=pos_sb[:], op=AG.mult)
            nc.vector.tensor_tensor(out=f_sb[:, :, o + 3:o + 5], in0=ds[:, 0:nslots, 4:6],
                                    in1=tmp[:], op=AG.subtract)
            nc.vector.tensor_copy(out=f_sb[:, :, o + 5:o + 6], in_=ds[:, 0:nslots, 6:7])
            nc.vector.memset(f_sb[:, :, o + 6:C], 0.0)

            if DEBUG:
                fdbg = sb.tile([128, nslots, C], dt.float32, tag="fdbg")
                nc.vector.tensor_copy(out=fdbg[:], in_=f_sb[:])
                nc.sync.dma_start(out=dbg["dbg_f"], in_=fdbg[:])

            # ---- Phase 3: x0 = f@G + g0 (via 32x32 stream transpose)
            fT = sb.tile([128, nslots, C], dt.bfloat16, tag="ft")
            nc.vector.transpose(out=fT[:], in_=f_sb[:])
            x0 = pp.tile([128, 64, C], dt.float32, tag="u")
            spb = 2048 // (C * 4)
            for s in range(nslots):
                for q in range(4):
                    nc.tensor.matmul(x0[32 * q:32 * q + 32, s, :],
                                     fT[32 * q:32 * q + 32, s, :],
                                     gmat[32 * q:32 * q + 32, :],
                                     start=(s % spb == 0), stop=True,
                                     skip_group_check=True,
                                     tile_position=(32 * q, 32 * q))
            x_sb = sb.tile([128, nslots, C], dt.bfloat16, tag="xsb")
            nc.vector.tensor_tensor(out=x_sb[:], in0=x0[:, 0:nslots, :], in1=g0b[:], op=AG.add)

            if DEBUG:
                xdbg = sb.tile([128, nslots, C], dt.float32, tag="fdbg")
                nc.vector.tensor_copy(out=xdbg[:], in_=x_sb[:])
                nc.sync.dma_start(out=dbg["dbg_x0"], in_=xdbg[:])

            # ---- Phase 4: GCN layers
            xT = sb.tile([128, nslots, C], dt.bfloat16, tag="xt")
            for l in range(cfg.L):
                nc.vector.transpose(out=xT[:], in_=x_sb[:])
                u = pp.tile([128, 64, C], dt.float32, tag="u")
                for s in range(nslots):
                    for q in range(4):
                        nc.tensor.matmul(u[32 * q:32 * q + 32, s, :],
                                         xT[32 * q:32 * q + 32, s, :],
                                         wk(l, cfg.TAPS, q),
                                         start=(s % spb == 0), stop=True,
                                         skip_group_check=True,
                                         tile_position=(32 * q, 32 * q))
                for k in range(cfg.TAPS - 1, -1, -1):
                    if DEBUG and l == 0 and k in (cfg.TAPS - 1, cfg.TAPS - 2):
                        nm = "dbg_u4" if k == cfg.TAPS - 1 else "dbg_u3"
                        udbg = sb.tile([128, nslots, C], dt.float32, tag="fdbg")
                        nc.vector.tensor_copy(out=udbg[:], in_=u[:, 0:nslots, :])
                        nc.sync.dma_start(out=dbg[nm], in_=udbg[:])
                    nc.vector.tensor_copy(out=ubf[:, :, 0:C],
                                          in_=u[:, 0:nslots, :])
                    nc.sync.dma_start(
                        out=ush.rearrange("(s p) c -> p s c", p=128),
                        in_=ubf[:])
                    if NO_COLLECTIVE:
                        nc.sync.dma_start(out=xtab[0:cfg.NSP], in_=ush[:])
                    else:
                        nc.gpsimd.collective_compute(
                            "AllGather", AG.bypass,
                            replica_groups=[list(range(cfg.NCORES))],
                            ins=[ush[:]], outs=[xtab[:]])
                    if DEBUG and l == 0 and k == cfg.TAPS - 1:
                        nc.sync.dma_start(out=dbg["dbg_xt"], in_=xtab[:])
                    u2 = pp.tile([128, 64, C], dt.float32, tag="u")
                    for s in range(nslots):
                        for q in range(4):
                            nc.tensor.matmul(u2[32 * q:32 * q + 32, s, :],
                                             xT[32 * q:32 * q + 32, s, :],
                                             wk(l, k, q),
                                             start=(s % spb == 0), stop=True,
                                             skip_group_check=True,
                                             tile_position=(32 * q, 32 * q))
                    for (t0, tn, half) in chunks:
                        ysb = sb.tile([128, cfg.GCHUNK, ROWP], dt.bfloat16,
                                      tag="ysb")
                        src = xtab[0:min(IDX_SPLIT, NTAB)] if half == 0 \
                            else xtab[IDX_SPLIT:NTAB]
                        nc.gpsimd.dma_gather(
                            out_ap=ysb[:, 0:tn, :],
                            in_ap=src,
                            idxs_ap=gidx[:, t0 * 8:(t0 + tn) * 8],
                            num_idxs=tn * KT,
                            num_idxs_reg=tn * KT,
                            elem_size=ROWP,
                            single_packet=False)
                        if DEBUG and l == 0 and k == cfg.TAPS - 1 and t0 == 0:
                            nc.sync.dma_start(out=dbg["dbg_y"], in_=ysb[:])
                        for ti in range(tn):
                            wg = int(widG[t0 + ti])
                            nc.tensor.matmul(
                                win_ap(u2, wg, C),
                                g_oh[:, t0 + ti], ysb[:, ti, 0:C],
                                start=False, stop=True, skip_group_check=True,
                                tile_position=(0, 32 * (wg % 4)))
                    u = u2
                # x = lrelu(u + b)
                vt = sb.tile([128, nslots, C], dt.float32, tag="vt")
                nc.vector.tensor_tensor(
                    out=vt[:], in0=u[:, 0:nslots, :],
                    in1=bb[:, l * nslots:(l + 1) * nslots, :], op=AG.add)
                vs = sb.tile([128, nslots, C], dt.float32, tag="vs")
                nc.vector.tensor_scalar(out=vs[:], in0=vt[:], scalar1=0.01,
                                        scalar2=None, op0=AG.mult)
                nc.vector.tensor_tensor(out=x_sb[:], in0=vt[:], in1=vs[:],
                                        op=AG.max)

            # ---- Phase 5: readout
            nc.vector.transpose(out=xT[:], in_=x_sb[:])
            op = pp.tile([128, 64, C], dt.float32, tag="u")
            for s in range(nslots):
                for q in range(4):
                    nc.tensor.matmul(op[32 * q:32 * q + 32, s, 0:OUT],
                                     xT[32 * q:32 * q + 32, s, :],
                                     wout[32 * q:32 * q + 32, :],
                                     start=(s % spb == 0), stop=True,
                                     skip_group_check=True,
                                     tile_position=(32 * q, 32 * q))
            osb = sb.tile([128, nslots, OUT], dt.float32, tag="osb")
            nc.vector.tensor_tensor(out=osb[:], in0=op[:, 0:nslots, 0:OUT], in1=rbb[:], op=AG.add)
            nc.sync.dma_start(out=out_sh[:], in_=osb[:])

    nc.compile()
    return nc


def make_in_maps(cfg, prep):
    consts = prep["consts"]
    in_maps = []
    for c in range(cfg.NCORES):
        m = dict(prep["cores"][c])
        m.update(consts)
        in_maps.append(m)
    return in_maps


def assemble_output(cfg, results):
    out = np.zeros((cfg.N, cfg.OUT), np.float32)
    for c in range(cfg.NCORES):
        sh = results[c]["out_sh"]          # [128, nslots, OUT]
        full = sh.transpose(1, 0, 2).reshape(cfg.NSP, cfg.OUT)
        out[c * cfg.NS:(c + 1) * cfg.NS] = full[:cfg.NS]
    return out


TRACE = False
LAST = {}


def kernel(**inputs):
    from concourse.bass_utils import run_bass_kernel_spmd
    cfg = CFG
    prep = _prep(cfg, inputs)
    nc = build_program(cfg, prep)
    res = run_bass_kernel_spmd(nc, make_in_maps(cfg, prep),
                               list(range(cfg.NCORES)), trace=TRACE)
    LAST["res"] = res
    return assemble_output(cfg, res.results)
